# revision 8
# baseline (speedup 1.0000x reference)
"""Trainium2 Bass kernel for GraphUnpoolingMesh.

reference:
    mid   = 0.5 * (x[src] + x[dst])            # [E, F] random-row gather
    new_x = concat([x, mid], axis=0)           # [N+E, F]
    new_ei row0 = [src | src | iotaA | tri0]   # iotaA = N + arange(E), tri0 == iotaA
    new_ei row1 = [dst | iotaA | dst | tri1]   # tri1 = N + p + [1,1,-2][p%3]

Strategy: data-parallel over edges across 8 NeuronCores, x replicated in each
core's HBM.  Each core computes mid for its 75000-edge shard with chunked
indirect-DMA gathers (K=128*ncol rows per instruction, int32 indices resident
in SBUF), fusing the dst gather with a CCE add, scaling by 0.5 on the scalar
engine, and storing contiguously.  new_ei segments are emitted on-device as
plain DMA copies of the input index slices and precomputed iota constant
inputs.  Host does only sharding, concatenation and dtype-preserving views.
"""

import numpy as np

# Problem shape (hardcoded per contract).
N, F, E = 100000, 128, 600000
NCORES = 8
ES = E // NCORES  # 75000 edges per core
NCOL_CHUNK = 1
ES_PAD = ((ES + 127) // 128) * 128  # 75008
COLS = ES_PAD // 128  # 586


def _chunks_for(cols: int, ncol: int):
    """Split `cols` SBUF index columns into chunks of <= ncol columns."""
    out = []
    c0 = 0
    while c0 < cols:
        out.append((c0, min(ncol, cols - c0)))
        c0 += ncol
    return out


def build_program(n, f, es, es_pad, ncol_chunk, n_devices=NCORES, fuse_add=True):
    """Build (and compile) the per-core Bass program.

    n: node count; f: feature dim; es: shard edge count; es_pad: padded to a
    multiple of 128; ncol_chunk: index columns (=128 edges each) per indirect
    gather.
    """
    import concourse.bacc as bacc
    import concourse.mybir as mybir
    from concourse import bass
    from concourse.tile import TileContext

    cols = es_pad // 128
    chunks = _chunks_for(cols, ncol_chunk)
    nc = bacc.Bacc(
        "TRN2",
        target_bir_lowering=False,
        debug=False,
        enable_asserts=False,
        num_devices=n_devices,
    )

    dt = mybir.dt
    x = nc.dram_tensor("x", [n, f], dt.float32, kind="ExternalInput")
    src_idx = nc.dram_tensor("src_idx", [128, cols], dt.int32, kind="ExternalInput")
    dst_idx = nc.dram_tensor("dst_idx", [128, cols], dt.int32, kind="ExternalInput")
    iota_a = nc.dram_tensor("iota_a", [es], dt.int32, kind="ExternalInput")
    iota_t = nc.dram_tensor("iota_t", [es], dt.int32, kind="ExternalInput")
    src_flat = nc.dram_tensor("src_flat", [es], dt.int32, kind="ExternalInput")
    dst_flat = nc.dram_tensor("dst_flat", [es], dt.int32, kind="ExternalInput")

    mid = nc.dram_tensor("mid", [es_pad, f], dt.float32, kind="ExternalOutput")
    ei_r0 = nc.dram_tensor("ei_r0", [4, es], dt.int32, kind="ExternalOutput")
    ei_r1 = nc.dram_tensor("ei_r1", [4, es], dt.int32, kind="ExternalOutput")

    with TileContext(nc) as tc:
        with (
            tc.tile_pool(name="idxp", bufs=24) as idxp,
            tc.tile_pool(name="datp", bufs=12) as datp,
        ):
            # new_ei segments: pure DRAM->DRAM byte copies.
            nc.sync.dma_start(out=ei_r0.ap()[0, :], in_=src_flat.ap()[:])
            nc.sync.dma_start(out=ei_r0.ap()[1, :], in_=src_flat.ap()[:])
            nc.sync.dma_start(out=ei_r0.ap()[2, :], in_=iota_a.ap()[:])
            nc.sync.dma_start(out=ei_r0.ap()[3, :], in_=iota_a.ap()[:])
            nc.sync.dma_start(out=ei_r1.ap()[0, :], in_=dst_flat.ap()[:])
            nc.sync.dma_start(out=ei_r1.ap()[1, :], in_=iota_a.ap()[:])
            nc.sync.dma_start(out=ei_r1.ap()[2, :], in_=dst_flat.ap()[:])
            nc.sync.dma_start(out=ei_r1.ap()[3, :], in_=iota_t.ap()[:])

            for col0, ncol in chunks:
                # Offset tables must be contiguous full tiles: the SWDGE
                # reads them linearly and ignores partition stride, so a
                # strided slice of a wide tile maps indices wrongly on HW.
                sidx = idxp.tile([128, ncol], dt.int32, tag="sidx")
                didx = idxp.tile([128, ncol], dt.int32, tag="didx")
                nc.sync.dma_start(out=sidx[:], in_=src_idx.ap()[:, col0 : col0 + ncol])
                nc.sync.dma_start(out=didx[:], in_=dst_idx.ap()[:, col0 : col0 + ncol])
                a = datp.tile([128, ncol * f], dt.float32, tag="a")
                nc.gpsimd.indirect_dma_start(
                    out=a[:],
                    out_offset=None,
                    in_=x.ap()[:, :],
                    in_offset=bass.IndirectOffsetOnAxis(ap=sidx[:], axis=0),
                )
                if fuse_add:
                    nc.gpsimd.indirect_dma_start(
                        out=a[:],
                        out_offset=None,
                        in_=x.ap()[:, :],
                        in_offset=bass.IndirectOffsetOnAxis(ap=didx[:], axis=0),
                        compute_op=mybir.AluOpType.add,
                    )
                else:
                    b = datp.tile([128, ncol * f], dt.float32, tag="b")
                    nc.gpsimd.indirect_dma_start(
                        out=b[:],
                        out_offset=None,
                        in_=x.ap()[:, :],
                        in_offset=bass.IndirectOffsetOnAxis(ap=didx[:], axis=0),
                    )
                    nc.vector.tensor_add(out=a[:], in0=a[:], in1=b[:])
                nc.scalar.mul(a[:], a[:], 0.5)
                out_ap = mid.ap()[col0 * 128 : (col0 + ncol) * 128, :].rearrange(
                    "(p c) f -> p c f", p=128
                )
                in_ap = a[:].rearrange("p (c f) -> p c f", f=f)
                nc.sync.dma_start(out=out_ap, in_=in_ap)

    nc.compile()
    return nc


def _pack_idx(idx_flat_pad, cols, ncol_chunk):
    """Arrange flat shard indices into the [128, cols] SBUF layout where
    chunk (col0, ncol) holds edge i = 128*col0 + p*ncol + j at [p, col0+j]."""
    out = np.empty((128, cols), dtype=np.int32)
    for col0, ncol in _chunks_for(cols, ncol_chunk):
        block = idx_flat_pad[128 * col0 : 128 * (col0 + ncol)].reshape(128, ncol)
        out[:, col0 : col0 + ncol] = block
    return out


def kernel(x, edge_index, _trace=False, _fuse=False):
    x = np.ascontiguousarray(np.asarray(x), dtype=np.float32)
    edge_index = np.asarray(edge_index)
    idt = edge_index.dtype
    assert idt in (np.int32, np.int64), idt

    from concourse.bass_utils import run_bass_kernel_spmd

    nc = build_program(N, F, ES, ES_PAD, NCOL_CHUNK, fuse_add=_fuse)

    in_maps = []
    T = -(-(E - 2) // 3)
    assert 3 * T == E
    pat = np.tile(np.array([1, 1, -2], dtype=np.int32), ES // 3)
    pad = np.zeros(ES_PAD - ES, np.int32)
    for c in range(NCORES):
        sh = slice(c * ES, (c + 1) * ES)
        src = np.ascontiguousarray(edge_index[0, sh], dtype=np.int32)
        dst = np.ascontiguousarray(edge_index[1, sh], dtype=np.int32)
        iota = (N + c * ES + np.arange(ES)).astype(np.int32)
        in_maps.append(
            {
                "x": x,
                "src_idx": _pack_idx(np.concatenate([src, pad]), COLS, NCOL_CHUNK),
                "dst_idx": _pack_idx(np.concatenate([dst, pad]), COLS, NCOL_CHUNK),
                "iota_a": iota,
                "iota_t": iota + pat,
                "src_flat": src,
                "dst_flat": dst,
            }
        )

    res = run_bass_kernel_spmd(nc, in_maps, core_ids=list(range(NCORES)))
    if _trace:
        # no NTFF hook under this axon container: warm wall-clock (includes
        # H2D/D2H of ~460MB; consistent across variants for A/B)
        import time as _time
        t0 = _time.time()
        res = run_bass_kernel_spmd(nc, in_maps, core_ids=list(range(NCORES)))
        dt_ns = (_time.time() - t0) * 1e9
        print(f"HW exec time: {dt_ns:.0f} ns (warm wall-clock incl. transfers)")

    mids = [r["mid"][:ES] for r in res.results]
    new_x = np.concatenate([x] + mids, axis=0)

    r0 = [r["ei_r0"] for r in res.results]
    r1 = [r["ei_r1"] for r in res.results]
    row0 = np.concatenate([r0[c][s] for s in range(4) for c in range(NCORES)])
    row1 = np.concatenate([r1[c][s] for s in range(4) for c in range(NCORES)])
    new_ei = np.stack([row0, row1], axis=0).astype(idt, copy=False)
    return new_x, new_ei
